# revision 1
# baseline (speedup 1.0000x reference)
"""Multi-head attention (ESIM-style masked softmax) on 8 trn2 NeuronCores.

Sharding: core c -> (batch b = c//2, head-group g = c%2). Each core runs
batch b with 8 of the 16 heads: Q/K/V projections restricted to its 512
channels (Wq/Wk/Wv column shards), attention, and a partial output
projection with its 512 rows of Wo. Host sums the two partials per batch.

On-device layout is fully "transposed" so no on-device transposes are
needed: host passes q^T/k^T/v^T; scores are computed as S^T[k,q]; the key
mask enters via zeroed V rows and a km-weighted denominator matmul; exp is
a single ACT op per tile; P^T feeds P·V directly as the moving operand.
All matmuls run in float32r (1 cycle/row at N>=512, ~1.5e-4 precision).
"""
import sys

for _p in ("/opt/trn_rl_repo",):
    if _p not in sys.path:
        sys.path.insert(0, _p)

import numpy as np

import concourse.bass as bass
import concourse.tile as tile
from concourse import mybir
from concourse.bass_utils import run_bass_kernel_spmd

# ---------------------------------------------------------------------------
# Workaround for this container's walrus build: it accepts at most ONE sem
# wait per lowered instruction. Split excess waits onto injected nops on the
# same (in-order) engine queue, and do the same for the kernel-tail drain.
# ---------------------------------------------------------------------------
import bass_rust
import concourse.tile as tile_mod
from concourse.vector_clock import ScopedClock

_MAX_WAITS = 1
_N_CARRIERS = 32
_wsplit_counter = [0]


def _patched_drain_and_barrier(self, tick_clock, wait_clock):
    nc = self.nc
    pre = [nc.sync.drain() for _ in range(_N_CARRIERS)]
    drain_inst = nc.sync.drain()
    wait_clock.add_sem_waits(
        drain_inst.ins, ScopedClock({None: tick_clock.global_clock})
    )
    si = drain_inst.ins.sync_info
    waits = list(si.on_wait) if si is not None else []
    if len(waits) > _MAX_WAITS:
        chunks = [waits[i : i + _MAX_WAITS] for i in range(0, len(waits), _MAX_WAITS)]
        *head, tail = chunks
        assert len(head) <= len(pre), f"too many drain waits: {len(waits)}"
        for inst, chunk in zip(pre, head):
            inst.ins.sync_info = bass_rust.SyncInfo(on_wait=chunk, on_update=[])
        drain_inst.ins.sync_info = bass_rust.SyncInfo(
            on_wait=tail, on_update=list(si.on_update) if si else []
        )
    nc.all_engine_barrier()
    assert self.sems is not None
    popped = nc._tile_sem_poison_stack.pop()
    assert popped is self._sem_poison
    nc.clear_and_free_semaphores(list(self.sems.allocated().values()))
    nc.all_engine_barrier()


def _split_excess_waits(nc, max_waits=_MAX_WAITS):
    n_split = 0
    for fn in nc.m.functions:
        for blk in fn.blocks:
            insts = blk.instructions
            if not any(
                inst.sync_info is not None
                and len(inst.sync_info.on_wait) > max_waits
                for inst in insts
            ):
                continue
            new = []
            for inst in insts:
                si = inst.sync_info
                waits = list(si.on_wait) if si is not None and si.on_wait else []
                if len(waits) > max_waits:
                    head, tail = waits[:-max_waits], waits[-max_waits:]
                    for w in head:
                        _wsplit_counter[0] += 1
                        nop = mybir.InstNoOp(
                            name=f"wsplit-{_wsplit_counter[0]}", ins=[], outs=[]
                        )
                        nop.engine = inst.engine
                        nop.sync_info = bass_rust.SyncInfo(on_wait=[w], on_update=[])
                        new.append(nop)
                        n_split += 1
                    inst.sync_info = bass_rust.SyncInfo(
                        on_wait=tail, on_update=list(si.on_update)
                    )
                new.append(inst)
            insts[:] = new
    return n_split


_orig_tile_exit = tile_mod.TileContext.__exit__


def _patched_tile_exit(self, *args, **kwargs):
    ret = _orig_tile_exit(self, *args, **kwargs)
    _split_excess_waits(self.nc)
    return ret


if getattr(tile_mod.TileContext, "_attn_patch", None) is None:
    tile_mod.TileContext._drain_and_barrier = _patched_drain_and_barrier
    tile_mod.TileContext.__exit__ = _patched_tile_exit
    tile_mod.TileContext._attn_patch = True

# ---------------------------------------------------------------------------
# Program constants
# ---------------------------------------------------------------------------
f32 = mybir.dt.float32
f32r = mybir.dt.float32r
AF = mybir.ActivationFunctionType
ALU = mybir.AluOpType

B, L, D = 4, 1024, 1024
CH = 512          # channels per core (8 heads x dh=64)
NHP = 4           # head pairs per core
DC = 8            # d (contraction) chunks of 128
KC = 8            # key-position chunks of 128
LC = 8            # l (query/row) chunks of 128
N_CORES = 8
SCALE = 0.125     # 1/sqrt(dh)


def build_program(phase=3, nonorm=False):
    nc = bass.Bass(trn_type="TRN2", target_bir_lowering=False, debug=False)

    qT_d = nc.dram_tensor("qT", [D, L], f32r, kind="ExternalInput").ap()
    kT_d = nc.dram_tensor("kT", [D, L], f32r, kind="ExternalInput").ap()
    vT_d = nc.dram_tensor("vT", [D, L], f32r, kind="ExternalInput").ap()
    wq_d = nc.dram_tensor("wq", [D, CH], f32r, kind="ExternalInput").ap()
    wk_d = nc.dram_tensor("wk", [D, CH], f32r, kind="ExternalInput").ap()
    wv_d = nc.dram_tensor("wv", [D, CH], f32r, kind="ExternalInput").ap()
    wo_d = nc.dram_tensor("wo", [CH, D], f32r, kind="ExternalInput").ap()
    km_d = nc.dram_tensor("km", [128, KC], f32, kind="ExternalInput").ap()
    qm_d = nc.dram_tensor("qm", [128, LC], f32, kind="ExternalInput").ap()
    out_d = nc.dram_tensor("out", [L, D], f32, kind="ExternalOutput").ap()
    scr_d = nc.dram_tensor("scr", [16, 1, 512], f32).ap()

    from contextlib import ExitStack
    with tile.TileContext(nc) as tc:
        with (
            tc.tile_pool(name="persist", bufs=1) as pers,
            tc.tile_pool(name="work", bufs=3) as work,
            ExitStack() as ctxstack,
        ):
            # ---- persistent SBUF tiles ----
            wo_t = pers.tile([128, 4 * 1024], f32r, tag="wo")
            km_t = pers.tile([128, KC], f32, tag="km")
            qm_t = pers.tile([128, LC], f32, tag="qm")
            QT_t = pers.tile([128, 4 * 1024], f32r, tag="QT")
            KT_t = pers.tile([128, 4 * 1024], f32r, tag="KT")
            V_t = pers.tile([128, KC * 520], f32r, tag="V")
            OT_ts = [pers.tile([128, 1024], f32r, tag=f"OT{i}", name=f"OT{i}")
                     for i in range(4)]

            nc.gpsimd.dma_start(km_t[:], km_d)
            nc.gpsimd.dma_start(qm_t[:], qm_d)
            wo3 = wo_d.rearrange("(c p) n -> c p n", p=128)
            for ci in range(4):
                nc.gpsimd.dma_start(wo_t[:, ci * 1024:(ci + 1) * 1024], wo3[ci])

            def load_w(pool, dram):
                t = pool.tile([128, DC * 512], f32r, tag=dram.tensor.name + "_t")
                s3 = dram.rearrange("(d p) n -> d p n", p=128)
                for d in range(DC):
                    nc.sync.dma_start(t[:, d * 512:(d + 1) * 512], s3[d])
                return t

            oaccp = ctxstack.enter_context(tc.tile_pool(name="oacc", bufs=1))
            # ---- projections (d-outer accumulation, 8 psum banks) ----
            with (
                tc.tile_pool(name="psP", bufs=8, space="PSUM") as psP,
                tc.tile_pool(name="pin", bufs=6) as pin,
            ):
                with tc.tile_pool(name="wpool_v", bufs=1) as wpool_v:
                    wv_t = load_w(wpool_v, wv_d)
                    v3 = vT_d.rearrange("(d p) l -> d p l", p=128)
                    psv = [psP.tile([128, 512], f32, tag="ps", name=f"psv{i}") for i in range(KC)]
                    for d in range(DC):
                        vt_c = pin.tile([128, L], f32r, tag="pin")
                        nc.sync.dma_start(vt_c[:], v3[d])
                        for ki in range(KC):
                            nc.tensor.matmul(
                                psv[ki][:],
                                vt_c[:, ki * 128:(ki + 1) * 128],
                                wv_t[:, d * 512:(d + 1) * 512],
                                start=(d == 0), stop=(d == DC - 1),
                            )
                    for ki in range(KC):
                        blk = V_t[:, ki * 520:(ki + 1) * 520].rearrange(
                            "p (h c) -> p h c", c=65)
                        nc.vector.tensor_scalar(
                            blk[:, :, 0:64],
                            psv[ki][:].rearrange("p (h c) -> p h c", c=64),
                            km_t[:, ki:ki + 1], None, ALU.mult,
                        )
                        nc.vector.tensor_copy(
                            blk[:, :, 64:65],
                            km_t[:, ki:ki + 1][:, None, :].to_broadcast((128, 8, 1)),
                        )

                # KT = (k @ Wk)^T -> KT_t[c, l], c-chunk ci at cols ci*1024
                def proj_T(w_dram, x3, dst):
                    w_t = load_w(wpool_qk, w_dram)
                    psg = [psP.tile([128, 512], f32, tag="ps", name=f"psg{i}") for i in range(8)]
                    for d in range(DC):
                        x_c = pin.tile([128, L], f32r, tag="pin")
                        nc.sync.dma_start(x_c[:], x3[d])
                        for ci in range(4):
                            for lh in range(2):
                                nc.tensor.matmul(
                                    psg[ci * 2 + lh][:],
                                    w_t[:, d * 512 + ci * 128: d * 512 + (ci + 1) * 128],
                                    x_c[:, lh * 512:(lh + 1) * 512],
                                    start=(d == 0), stop=(d == DC - 1),
                                )
                    for ci in range(4):
                        for lh in range(2):
                            nc.vector.tensor_copy(
                                dst[:, ci * 1024 + lh * 512: ci * 1024 + (lh + 1) * 512],
                                psg[ci * 2 + lh][:],
                            )

                k3 = kT_d.rearrange("(d p) l -> d p l", p=128)
                q3 = qT_d.rearrange("(d p) l -> d p l", p=128)
                with tc.tile_pool(name="wpool_qk", bufs=1) as wpool_qk:
                    proj_T(wk_d, k3, KT_t)
                    proj_T(wq_d, q3, QT_t)

            if phase == 1:
                o4 = out_d.rearrange("(a p) n -> a p n", p=128)
                for j in range(4):
                    nc.sync.dma_start(o4[j], V_t[:, j * 1024:(j + 1) * 1024].bitcast(f32))
                return nc

            # ---- attention: one head at a time, double-buffered ST ----
            with (
                tc.tile_pool(name="psA", bufs=2, space="PSUM") as psA,
                tc.tile_pool(name="psST", bufs=2, space="PSUM") as psST,
                tc.tile_pool(name="psPP", bufs=2, space="PSUM") as psPP,
            ):
                oacc = {}
                for h in range(8):
                    hp, ho = h // 2, (h % 2) * 64
                    co = hp * 1024
                    rows = slice(ho, ho + 64)
                    for qh in range(2):
                        qsl = slice(co + qh * 512, co + (qh + 1) * 512)
                        qs2 = slice(qh * 512, (qh + 1) * 512)
                        u = psA.tile([65, 512], f32, tag="u",
                                     name=f"u_{h}_{qh}")
                        for ki2 in range(4):
                            st = psST.tile([128, 1024], f32, tag="st",
                                           name=f"st_{h}_{qh}_{ki2}")
                            for kk in range(2):
                                ki = 2 * ki2 + kk
                                ksl = slice(co + ki * 128, co + (ki + 1) * 128)
                                nc.tensor.matmul(
                                    st[:, kk * 512:(kk + 1) * 512],
                                    KT_t[rows, ksl], QT_t[rows, qsl],
                                    start=True, stop=True,
                                )
                            et = work.tile([128, 1024], f32r, tag="et",
                                           name=f"et_{h}_{qh}_{ki2}")
                            nc.scalar.activation(et[:], st[:], AF.Exp, scale=SCALE)
                            for kk in range(2):
                                ki = 2 * ki2 + kk
                                off = ki * 520 + h * 65
                                nc.tensor.matmul(
                                    u[:], V_t[:, off:off + 65],
                                    et[:, kk * 512:(kk + 1) * 512],
                                    start=(ki == 0), stop=(ki == KC - 1),
                                )
                        if nonorm:
                            nc.vector.tensor_copy(OT_ts[hp][rows, qs2], u[0:64, :])
                        else:
                            r0 = work.tile([1, 512], f32, tag="r0",
                                           name=f"r0_{h}_{qh}")
                            nc.vector.reciprocal(r0[:], u[64:65, :])
                            pi = h * 2 + qh
                            nc.sync.dma_start(scr_d[pi][0:1, :], r0[:])
                            rb = work.tile([64, 512], f32, tag="rb",
                                           name=f"rb_{h}_{qh}")
                            src = scr_d[pi, 0, :]
                            bcast = bass.AP(
                                tensor=src.tensor, offset=src.offset,
                                ap=[[0, 64]] + list(src.ap),
                            )
                            nc.gpsimd.dma_start(rb[:], bcast)
                            nc.vector.tensor_tensor(
                                OT_ts[hp][rows, qs2], u[0:64, :], rb[:], ALU.mult
                            )
                    if h % 2 == 1:
                        for li in range(LC):
                            for oh in range(2):
                                po = psPP.tile([128, 512], f32, tag="pp",
                                               name=f"po_{hp}_{li}_{oh}")
                                nc.tensor.matmul(
                                    po[:],
                                    OT_ts[hp][:, li * 128:(li + 1) * 128],
                                    wo_t[:, hp * 1024 + oh * 512: hp * 1024 + (oh + 1) * 512],
                                    start=True, stop=True,
                                )
                                if hp == 0:
                                    t = oaccp.tile(
                                        [128, 512], f32, tag=f"oacc{li}_{oh}",
                                        name=f"oacc{li}_{oh}_t")
                                    oacc[(li, oh)] = t
                                    nc.vector.tensor_copy(t[:], po[:])
                                else:
                                    t = oacc[(li, oh)]
                                    nc.vector.tensor_tensor(
                                        t[:], po[:], t[:], ALU.add
                                    )

            if phase == 2:
                o4 = out_d.rearrange("(a p) n -> a p n", p=128)
                for j in range(4):
                    nc.sync.dma_start(o4[j], OT_ts[j][:].bitcast(f32))
                return nc

            # ---- finalize: qm scale + DMA out ----
            for li in range(LC):
                for oh in range(2):
                    ob = work.tile([128, 512], f32, tag="ob",
                                   name=f"ob_{li}_{oh}")
                    nc.scalar.activation(
                        ob[:], oacc[(li, oh)][:], AF.Copy,
                        scale=qm_t[:, li:li + 1],
                    )
                    nc.sync.dma_start(
                        out_d[li * 128:(li + 1) * 128, oh * 512:(oh + 1) * 512],
                        ob[:],
                    )
    return nc


_cache = {}


def _get_program():
    if "nc" not in _cache:
        _cache["nc"] = build_program()
    return _cache["nc"]


def kernel(query, key, value, query_mask, key_mask, Wq, Wk, Wv, Wo):
    query = np.asarray(query, dtype=np.float32)
    key = np.asarray(key, dtype=np.float32)
    value = np.asarray(value, dtype=np.float32)
    Wq = np.asarray(Wq, dtype=np.float32)
    Wk = np.asarray(Wk, dtype=np.float32)
    Wv = np.asarray(Wv, dtype=np.float32)
    Wo = np.asarray(Wo, dtype=np.float32)

    nc = _get_program()

    qT = [np.ascontiguousarray(query[b].T) for b in range(B)]
    kT = [np.ascontiguousarray(key[b].T) for b in range(B)]
    vT = [np.ascontiguousarray(value[b].T) for b in range(B)]
    km = [
        np.ascontiguousarray(key_mask[b].astype(np.float32).reshape(KC, 128).T)
        for b in range(B)
    ]
    qm = [
        np.ascontiguousarray(query_mask[b].astype(np.float32).reshape(LC, 128).T)
        for b in range(B)
    ]
    wq_g = [np.ascontiguousarray(Wq[:, g * CH:(g + 1) * CH]) for g in range(2)]
    wk_g = [np.ascontiguousarray(Wk[:, g * CH:(g + 1) * CH]) for g in range(2)]
    wv_g = [np.ascontiguousarray(Wv[:, g * CH:(g + 1) * CH]) for g in range(2)]
    wo_g = [np.ascontiguousarray(Wo[g * CH:(g + 1) * CH, :]) for g in range(2)]

    in_maps = []
    for c in range(N_CORES):
        b, g = c // 2, c % 2
        in_maps.append({
            "qT": qT[b], "kT": kT[b], "vT": vT[b],
            "wq": wq_g[g], "wk": wk_g[g], "wv": wv_g[g], "wo": wo_g[g],
            "km": km[b], "qm": qm[b],
        })

    res = run_bass_kernel_spmd(nc, in_maps, list(range(N_CORES)))
    out = np.empty((B, L, D), dtype=np.float32)
    for b in range(B):
        out[b] = res.results[2 * b]["out"] + res.results[2 * b + 1]["out"]
    return out



# revision 15
# speedup vs baseline: 2.3008x; 2.3008x over previous
"""Multi-head attention (ESIM-style masked softmax) on 8 trn2 NeuronCores.

Sharding: core c -> (batch b = c//2, head-group g = c%2). Each core runs
batch b with 8 of the 16 heads (512 channels): Wq/Wk/Wv column shards,
attention, and a partial output projection with its 512 rows of Wo. Host
sums the two partials per batch.

v2: host-side mask compaction (masked queries/keys contribute exactly 0,
so only the ~nq/nk surviving rows are shipped/computed, padded to a
multiple of 128), bf16 operands everywhere (PSUM accumulates fp32),
row-packed score matmuls (two dh=64 heads run concurrently in PE row
groups 0-1/2-3), wide exp tiles, PSUM-accumulated output projection, and
batched approx reciprocals instead of single-partition reciprocals.
"""
import sys

for _p in ("/opt/trn_rl_repo",):
    if _p not in sys.path:
        sys.path.insert(0, _p)

import numpy as np
import ml_dtypes

import concourse.bass as bass
import concourse.tile as tile
from concourse import mybir
from concourse.bass_utils import run_bass_kernel_spmd

# ---------------------------------------------------------------------------
# Workaround for this container's walrus build: it accepts at most ONE sem
# wait per lowered instruction. Split excess waits onto injected nops on the
# same (in-order) engine queue, and do the same for the kernel-tail drain.
# ---------------------------------------------------------------------------
import bass_rust
import concourse.tile as tile_mod
from concourse.vector_clock import ScopedClock

_MAX_WAITS = 1
_N_CARRIERS = 32
_wsplit_counter = [0]


def _patched_drain_and_barrier(self, tick_clock, wait_clock):
    nc = self.nc
    pre = [nc.sync.drain() for _ in range(_N_CARRIERS)]
    drain_inst = nc.sync.drain()
    wait_clock.add_sem_waits(
        drain_inst.ins, ScopedClock({None: tick_clock.global_clock})
    )
    si = drain_inst.ins.sync_info
    waits = list(si.on_wait) if si is not None else []
    if len(waits) > _MAX_WAITS:
        chunks = [waits[i : i + _MAX_WAITS] for i in range(0, len(waits), _MAX_WAITS)]
        *head, tail = chunks
        assert len(head) <= len(pre), f"too many drain waits: {len(waits)}"
        for inst, chunk in zip(pre, head):
            inst.ins.sync_info = bass_rust.SyncInfo(on_wait=chunk, on_update=[])
        drain_inst.ins.sync_info = bass_rust.SyncInfo(
            on_wait=tail, on_update=list(si.on_update) if si else []
        )
    nc.all_engine_barrier()
    assert self.sems is not None
    popped = nc._tile_sem_poison_stack.pop()
    assert popped is self._sem_poison
    nc.clear_and_free_semaphores(list(self.sems.allocated().values()))
    nc.all_engine_barrier()


def _split_excess_waits(nc, max_waits=_MAX_WAITS):
    n_split = 0
    for fn in nc.m.functions:
        for blk in fn.blocks:
            insts = blk.instructions
            if not any(
                inst.sync_info is not None
                and len(inst.sync_info.on_wait) > max_waits
                for inst in insts
            ):
                continue
            new = []
            for inst in insts:
                si = inst.sync_info
                waits = list(si.on_wait) if si is not None and si.on_wait else []
                if len(waits) > max_waits:
                    head, tail = waits[:-max_waits], waits[-max_waits:]
                    for w in head:
                        _wsplit_counter[0] += 1
                        nop = mybir.InstNoOp(
                            name=f"wsplit-{_wsplit_counter[0]}", ins=[], outs=[]
                        )
                        nop.engine = inst.engine
                        nop.sync_info = bass_rust.SyncInfo(on_wait=[w], on_update=[])
                        new.append(nop)
                        n_split += 1
                    inst.sync_info = bass_rust.SyncInfo(
                        on_wait=tail, on_update=list(si.on_update)
                    )
                new.append(inst)
            insts[:] = new
    return n_split


_orig_tile_exit = tile_mod.TileContext.__exit__


def _patched_tile_exit(self, *args, **kwargs):
    ret = _orig_tile_exit(self, *args, **kwargs)
    _split_excess_waits(self.nc)
    return ret


if getattr(tile_mod.TileContext, "_attn_patch", None) is None:
    tile_mod.TileContext._drain_and_barrier = _patched_drain_and_barrier
    tile_mod.TileContext.__exit__ = _patched_tile_exit
    tile_mod.TileContext._attn_patch = True

# ---------------------------------------------------------------------------
# Program constants
# ---------------------------------------------------------------------------
f32 = mybir.dt.float32
bf16 = mybir.dt.bfloat16
AF = mybir.ActivationFunctionType
ALU = mybir.AluOpType

B, L, D = 4, 1024, 1024
CH = 512          # channels per core (8 heads x dh=64)
DC = 8            # d (contraction) chunks of 128
N_CORES = 8
SCALE = 0.125     # 1/sqrt(dh)


def _chunks(n):
    """Split n columns into moving-operand chunks of <=512."""
    out = []
    off = 0
    while off < n:
        w = min(512, n - off)
        out.append((off, w))
        off += w
    return out


def build_program(NQ, NK):
    NQC, NKC = NQ // 128, NK // 128
    nc = bass.Bass(trn_type="TRN2", target_bir_lowering=False, debug=False)

    qT_d = nc.dram_tensor("qT", [D, NQ], bf16, kind="ExternalInput").ap()
    kT_d = nc.dram_tensor("kT", [D, NK], bf16, kind="ExternalInput").ap()
    vT_d = nc.dram_tensor("vT", [D, NK], bf16, kind="ExternalInput").ap()
    wq_d = nc.dram_tensor("wq", [D, CH], bf16, kind="ExternalInput").ap()
    wk_d = nc.dram_tensor("wk", [D, CH], bf16, kind="ExternalInput").ap()
    wv_d = nc.dram_tensor("wv", [D, CH], bf16, kind="ExternalInput").ap()
    wo_d = nc.dram_tensor("wo", [CH, D], bf16, kind="ExternalInput").ap()
    km_d = nc.dram_tensor("km", [128, NKC], bf16, kind="ExternalInput").ap()
    out_d = nc.dram_tensor("out", [NQ, D], f32, kind="ExternalOutput").ap()

    qcl = _chunks(NQ)

    with tile.TileContext(nc) as tc:
        with (
            tc.tile_pool(name="persist", bufs=1) as pers,
            tc.tile_pool(name="work", bufs=3) as work,
        ):
            # ---- persistent SBUF tiles ----
            km_t = pers.tile([128, NKC], bf16, tag="km")
            wo_t = pers.tile([128, 4 * 1024], bf16, tag="wo")
            QT_t = pers.tile([128, 4 * NQ], bf16, tag="QT")
            KT_t = pers.tile([128, 4 * NK], bf16, tag="KT")
            V_t = pers.tile([128, NKC * 512], bf16, tag="V")
            # km replicated 64x per ki chunk: stationary for the
            # denominator matmuls (one col-tiled MM per head).
            kmr_t = pers.tile([128, NKC * 64], bf16, tag="kmr")
            OT_ts = [pers.tile([128, NQ], bf16, tag=f"OT{i}", name=f"OT{i}")
                     for i in range(4)]
            # staging for full kT/qT/vT (bf16, 8 d-chunks each)
            k_sb = pers.tile([128, DC * NK], bf16, tag="k_sb")
            q_sb = pers.tile([128, DC * NQ], bf16, tag="q_sb")
            v_sb = pers.tile([128, DC * NK], bf16, tag="v_sb")

            # ---- input DMAs (one per tensor, spread across queues) ----
            nc.gpsimd.dma_start(km_t[:], km_d)
            k3 = kT_d.rearrange("(d p) l -> p d l", p=128)
            q3 = qT_d.rearrange("(d p) l -> p d l", p=128)
            v3 = vT_d.rearrange("(d p) l -> p d l", p=128)
            nc.sync.dma_start(
                k_sb[:].rearrange("p (d l) -> p d l", d=DC), k3)
            nc.gpsimd.dma_start(
                v_sb[:].rearrange("p (d l) -> p d l", d=DC), v3)
            nc.sync.dma_start(
                q_sb[:].rearrange("p (d l) -> p d l", d=DC), q3)
            wo3 = wo_d.rearrange("(c p) n -> p c n", p=128)
            nc.gpsimd.dma_start(
                wo_t[:].rearrange("p (c n) -> p c n", c=4), wo3)

            def load_w(pool, dram, eng):
                t = pool.tile([128, DC * 512], bf16, tag=dram.tensor.name + "_t")
                s3 = dram.rearrange("(d p) n -> p d n", p=128)
                eng.dma_start(t[:].rearrange("p (d n) -> p d n", d=DC), s3)
                return t

            with (
                tc.tile_pool(name="wpool", bufs=1) as wpool,
                tc.tile_pool(name="psP", bufs=4, space="PSUM") as psP,
            ):
                wv_t = load_w(wpool, wv_d, nc.gpsimd)
                wk_t = load_w(wpool, wk_d, nc.sync)
                wq_t = load_w(wpool, wq_d, nc.sync)

                # ---- V projection: ki-outer, accumulate over d ----
                # psv[k, ch] = sum_d vT[d, k] * wv[d, ch]
                for ki in range(NKC):
                    psv = psP.tile([128, 512], f32, tag="ps", name=f"psv{ki}")
                    for d in range(DC):
                        nc.tensor.matmul(
                            psv[:],
                            v_sb[:, d * NK + ki * 128: d * NK + (ki + 1) * 128],
                            wv_t[:, d * 512:(d + 1) * 512],
                            start=(d == 0), stop=(d == DC - 1),
                        )
                    nc.vector.tensor_copy(
                        V_t[:, ki * 512:(ki + 1) * 512], psv[:])
                    nc.vector.tensor_copy(
                        kmr_t[:, ki * 64:(ki + 1) * 64][:, None, :],
                        km_t[:, ki:ki + 1][:, None, :].to_broadcast((128, 1, 64)),
                    )

                # ---- K / Q projections: ci-outer, chunk-outer, accum d ----
                def proj(w_t, x_sb, dst, NX, xcl):
                    # dst[ci*128 + c, l] = sum_d w[d, ci*128+c] * x[d, l]
                    for ci in range(4):
                        for off, w in xcl:
                            ps = psP.tile([128, 512], f32, tag="ps",
                                          name=f"ps_{dst.name}_{ci}_{off}")
                            for d in range(DC):
                                nc.tensor.matmul(
                                    ps[:, 0:w],
                                    w_t[:, d * 512 + ci * 128: d * 512 + (ci + 1) * 128],
                                    x_sb[:, d * NX + off: d * NX + off + w],
                                    start=(d == 0), stop=(d == DC - 1),
                                )
                            nc.vector.tensor_copy(
                                dst[:, ci * NX + off: ci * NX + off + w],
                                ps[:, 0:w],
                            )

                proj(wk_t, k_sb, KT_t, NK, _chunks(NK))
                proj(wq_t, q_sb, QT_t, NQ, qcl)

            # ---- attention: pairs of heads, 2-ki-wide exp tiles ----
            # One (pair, qchunk) at a time. Score matmuls for the two heads
            # of a pair run concurrently in PE row groups 0-1 / 2-3.
            with (
                tc.tile_pool(name="psST", bufs=1, space="PSUM") as psST,
                tc.tile_pool(name="psU", bufs=2, space="PSUM") as psU,
                tc.tile_pool(name="psD", bufs=2, space="PSUM") as psD,
            ):
                KI2 = (NKC + 1) // 2
                steps = [(p, ci) for p in range(4) for ci in range(len(qcl))]
                pv_backlog = []   # deferred PV+normalize emitters
                for p, ciq in steps:
                    qoff, qN = qcl[ciq]
                    co = p * NK   # KT col offset for this pair
                    # 4 bank-aligned slots (kk, hh) at stride 512
                    st = psST.tile([128, 2048], f32, tag="st",
                                   name=f"st_{p}_{ciq}")
                    et_tiles = []
                    for ki2 in range(KI2):
                        nki = min(2, NKC - 2 * ki2)
                        # emit deferred PV work first: it is ready (its et
                        # exists) and keeps PE busy while ACT catches up
                        if pv_backlog:
                            pv_backlog.pop(0)()
                        for kk in range(nki):
                            ki = 2 * ki2 + kk
                            ksl = slice(co + ki * 128, co + (ki + 1) * 128)
                            for hh in range(2):
                                rows = slice(hh * 64, (hh + 1) * 64)
                                s = kk * 2 + hh
                                nc.tensor.matmul(
                                    st[:, s * 512: s * 512 + qN],
                                    KT_t[rows, ksl],
                                    QT_t[rows, p * NQ + qoff: p * NQ + qoff + qN],
                                    start=True, stop=True,
                                )
                        et = work.tile([128, 2048], bf16, tag="et",
                                       name=f"et_{p}_{ciq}_{ki2}", bufs=8)
                        st3 = st[:].rearrange("p (s c) -> p s c", c=512)
                        et3 = et[:].rearrange("p (s c) -> p s c", c=qN)
                        nc.scalar.activation(
                            et3[:, 0:nki * 2, :], st3[:, 0:nki * 2, 0:qN],
                            AF.Exp, scale=SCALE,
                        )
                        et_tiles.append(et)

                    def emit_pv(p=p, ciq=ciq, qoff=qoff, qN=qN,
                                et_tiles=et_tiles):
                        # PV for both heads of the pair in one full-density
                        # matmul per ki (stationary = the pair's 128 V cols);
                        # denominators via col-tiled M=64 matmuls against
                        # replicated km into one shared [128, qN] tile
                        # (head A -> partitions 0:64, head B -> 64:128).
                        u = psU.tile([128, 512], f32, tag="u",
                                     name=f"u_{p}_{ciq}")
                        dn = psD.tile([128, 512], f32, tag="dn",
                                      name=f"dn_{p}_{ciq}")
                        for ki in range(NKC):
                            et = et_tiles[ki // 2]
                            e0 = (ki % 2) * 2 * qN
                            for hh in range(2):
                                h = 2 * p + hh
                                esl = slice(e0 + hh * qN, e0 + (hh + 1) * qN)
                                nc.tensor.matmul(
                                    u[hh * 64:(hh + 1) * 64, 0:qN],
                                    V_t[:, ki * 512 + h * 64:
                                        ki * 512 + (h + 1) * 64],
                                    et[:, esl],
                                    start=(ki == 0), stop=(ki == NKC - 1),
                                    tile_position=(0, hh * 64),
                                )
                            for hh in range(2):
                                esl = slice(e0 + hh * qN, e0 + (hh + 1) * qN)
                                nc.tensor.matmul(
                                    dn[hh * 64:(hh + 1) * 64, 0:qN],
                                    kmr_t[:, ki * 64:(ki + 1) * 64],
                                    et[:, esl],
                                    start=(ki == 0), stop=(ki == NKC - 1),
                                    tile_position=(0, hh * 64),
                                )
                        rc = work.tile([128, 512], f32, tag="rc",
                                       name=f"rc_{p}_{ciq}")
                        nc.vector.reciprocal(rc[:, 0:qN], dn[:, 0:qN])
                        # u partitions = the pair's 128 channels
                        # (head A = 0:64, head B = 64:128), matching rc.
                        nc.vector.tensor_tensor(
                            OT_ts[p][:, qoff:qoff + qN],
                            u[:, 0:qN], rc[:, 0:qN], ALU.mult,
                        )

                    pv_backlog.append(emit_pv)
                for fn in pv_backlog:
                    fn()

            # ---- output projection: accumulate over head pairs in PSUM ----
            with tc.tile_pool(name="psO", bufs=3, space="PSUM") as psO:
                for li in range(NQC):
                    for oh in range(2):
                        po = psO.tile([128, 512], f32, tag="po",
                                      name=f"po_{li}_{oh}")
                        for hp in range(4):
                            nc.tensor.matmul(
                                po[:],
                                OT_ts[hp][:, li * 128:(li + 1) * 128],
                                wo_t[:, hp * 1024 + oh * 512:
                                     hp * 1024 + (oh + 1) * 512],
                                start=(hp == 0), stop=(hp == 3),
                            )
                        ob = work.tile([128, 512], f32, tag="ob",
                                       name=f"ob_{li}_{oh}")
                        nc.scalar.copy(ob[:], po[:])
                        nc.sync.dma_start(
                            out_d[li * 128:(li + 1) * 128,
                                  oh * 512:(oh + 1) * 512],
                            ob[:],
                        )
    return nc


_cache = {}


def _get_program(NQ, NK):
    key = (NQ, NK)
    if key not in _cache:
        _cache[key] = build_program(NQ, NK)
    return _cache[key]


def _pad_cap(n):
    return max(128, -(-n // 128) * 128)


def kernel(query, key, value, query_mask, key_mask, Wq, Wk, Wv, Wo,
           _trace=False):
    query = np.asarray(query, dtype=np.float32)
    key = np.asarray(key, dtype=np.float32)
    value = np.asarray(value, dtype=np.float32)
    query_mask = np.asarray(query_mask)
    key_mask = np.asarray(key_mask)
    Wq = np.asarray(Wq, dtype=np.float32)
    Wk = np.asarray(Wk, dtype=np.float32)
    Wv = np.asarray(Wv, dtype=np.float32)
    Wo = np.asarray(Wo, dtype=np.float32)

    qidx = [np.nonzero(query_mask[b])[0] for b in range(B)]
    kidx = [np.nonzero(key_mask[b])[0] for b in range(B)]
    NQ = _pad_cap(max(len(ix) for ix in qidx))
    NK = _pad_cap(max(len(ix) for ix in kidx))
    NKC = NK // 128

    nc = _get_program(NQ, NK)

    bf = ml_dtypes.bfloat16
    wq_g = [np.ascontiguousarray(Wq[:, g * CH:(g + 1) * CH]).astype(bf)
            for g in range(2)]
    wk_g = [np.ascontiguousarray(Wk[:, g * CH:(g + 1) * CH]).astype(bf)
            for g in range(2)]
    wv_g = [np.ascontiguousarray(Wv[:, g * CH:(g + 1) * CH]).astype(bf)
            for g in range(2)]
    wo_g = [np.ascontiguousarray(Wo[g * CH:(g + 1) * CH, :]).astype(bf)
            for g in range(2)]

    qT, kT, vT, km = [], [], [], []
    for b in range(B):
        qc = np.zeros((D, NQ), dtype=bf)
        qc[:, :len(qidx[b])] = query[b][qidx[b]].T
        qT.append(qc)
        kc = np.zeros((D, NK), dtype=bf)
        kc[:, :len(kidx[b])] = key[b][kidx[b]].T
        kT.append(kc)
        vc = np.zeros((D, NK), dtype=bf)
        vc[:, :len(kidx[b])] = value[b][kidx[b]].T
        vT.append(vc)
        kmv = np.zeros(NK, dtype=np.float32)
        kmv[:len(kidx[b])] = 1.0
        km.append(np.ascontiguousarray(
            kmv.reshape(NKC, 128).T.astype(bf)))

    in_maps = []
    for c in range(N_CORES):
        b, g = c // 2, c % 2
        in_maps.append({
            "qT": qT[b], "kT": kT[b], "vT": vT[b],
            "wq": wq_g[g], "wk": wk_g[g], "wv": wv_g[g], "wo": wo_g[g],
            "km": km[b],
        })

    res = run_bass_kernel_spmd(nc, in_maps, list(range(N_CORES)),
                               trace=_trace)
    out = np.zeros((B, L, D), dtype=np.float32)
    for b in range(B):
        part = res.results[2 * b]["out"] + res.results[2 * b + 1]["out"]
        out[b][qidx[b]] = part[:len(qidx[b])]
    if _trace:
        return out, res
    return out
